# revision 1
# baseline (speedup 1.0000x reference)
"""DeformConv1D Trainium2 Bass kernel.

Problem: B=64, C=64, L=4096, OUTC=128, KS=3 deformable conv1d.

Math (derived from the reference, validated in numpy):
    offset[b,k,t] = sum_{c,j} w_off[k,c,j] * xp[b,c,t+j] + b_off[k]
    p = t + k + offset[b,k,t]
    fl = round_half_even(p - 0.5)   (== floor(p) except at integer p, where coef=0)
    f  = p - fl
    coef = (1 <= p <= 4096) * f * (1 - f)
    out[b,o,t] = sum_{k,c} w_conv[o,c,k] * coef[b,t,k] * xs2[b,c,fl[b,t,k]]
where xp = zero-pad(x, 1) (length 4098) and xs2[u] = xp[u] + xp[u+1].
All masked / clipped / out-of-range cases collapse to coef == 0.

Device mapping (8 NeuronCores, data-parallel over batch, 8 samples/core).
Per core, per sample:
  - time axis folded in two halves across the 128 SBUF partitions:
    partition (64h + c); half h covers t_g in [2048h, 2048h + 2048) with
    xs2 stored locally as u_l = u_g - 2032*h, u_l in [0, 2065).
  - offset conv + main conv are float32r matmuls; the two halves use
    disjoint PE row groups (tile_position (0,0) vs (64,0)) and overlap.
  - per-(t,k) coef/idx elementwise chain on DVE in a [128, 96] layout:
    partition P = 64h + m, free = 32k + i, holding t_l = 32m + i
    (i = 16u + rr, u in {0,1}, rr in [0,16)).
  - gather runs on GPSIMD ap_gather with per-16-partition-core index
    lists in order q = 2048k + 1024u + 16m + rr; the wrapped index tile
    is built via one 16-bit XBAR DMA-transpose + 12 small SBUF DMAs.
  - coef is replicated over the 64 channel partitions by 0-stride DMAs
    (fp16), one DVE multiply scales the gathered data (in q order).
  - main conv consumes contiguous q-slices; the PSUM->SBUF copies
    un-permute q -> t so the output DMA is fully contiguous.
"""

import sys

import numpy as np

sys.path.insert(0, "/opt/trn_rl_repo")

import concourse.bass as bass
import concourse.bacc as bacc
import concourse.mybir as mybir
import concourse.tile as tile
from concourse.alu_op_type import AluOpType

N_CORES = 8
B, C, L, O, KS = 64, 64, 4096, 128, 3
BPC = B // N_CORES          # samples per core
TL = 2048                   # t_local per half
H1OFF = 2032                # xs2/xp global offset of half 1
NE = 2065                   # xs2 elements per half (local u in [0, NE))
NIDX = KS * TL              # 6144 gather indices per core-list
MAGIC = 12582912.0          # 1.5 * 2^23
F32 = mybir.dt.float32
F32R = mybir.dt.float32r
F16 = mybir.dt.float16
I16 = mybir.dt.int16

_PROGRAM = None


def _bc(ap, n):
    """Prepend a 0-stride broadcast dim of size n to an AP."""
    return bass.AP(ap.tensor, ap.offset, [[0, n]] + list(ap.ap))


def _emit_sample(nc, b, pools, consts, stage=9):
    (xpp, xs2p, gatp, crepp, smallp, outp, pso, psm) = pools
    (wo_r, wk_r, base_sb, hoff_sb, x_d, out_d, T_d, cf_d, cfq_d, list_d, wrD) = consts
    v = nc.vector

    # ---- 1. load folded padded input -------------------------------------
    # xp_fold[64h+c, u_l] = xp[c, u_l + H1OFF*h];  xp = [0, x[0..4095], 0]
    xp = xpp.tile([128, NE + 1], F32, name=f"xp{b}", tag="xp")
    v.memset(xp[0:64, 0:1], 0.0)
    v.memset(xp[64:128, NE : NE + 1], 0.0)
    nc.sync.dma_start(xp[0:64, 1 : NE + 1], x_d[b, :, 0:NE])
    nc.sync.dma_start(xp[64:128, 0:NE], x_d[b, :, H1OFF - 1 : H1OFF - 1 + NE])

    # ---- 2. xs2 ----------------------------------------------------------
    xs2 = xs2p.tile([128, NE], F32, name=f"xs2{b}", tag="xs2")
    v.tensor_add(xs2[:], xp[:, 0:NE], xp[:, 1 : NE + 1])

    if stage <= 1:
        nc.sync.dma_start(out_d[b, :, 0:2048], xs2[:, 0:2048].bitcast(F32))
        return

    # ---- 3. offset conv (float32r matmuls, halves on disjoint row groups)
    # fp32r matmul operands must be produced as float32r (walrus verifier):
    # one DVE convert pass of the padded input.
    xpr = xpp.tile([128, NE + 1], F32R, name=f"xpr{b}", tag="xpr")
    v.tensor_copy(xpr[:], xp[:])
    # offmA[kk, t_g] = offset[kk, t_g]
    offmA = smallp.tile([KS, L], F32, name=f"offmA{b}", tag="offmA", bufs=1)
    for tt in range(4):
        for h in range(2):
            ps = pso.tile([KS, 512], F32, name=f"psoff{b}_{tt}_{h}", tag=f"psoff{h}")
            pl = 64 * h
            cb = tt * 512 + (16 if h else 0)
            for j in range(KS):
                nc.tensor.matmul(
                    ps[:],
                    wo_r[pl : pl + 64, j, :],
                    xpr[pl : pl + 64, cb + j : cb + j + 512],
                    start=(j == 0),
                    stop=(j == KS - 1),
                    tile_position=(pl, 0),
                )
            tg = 2048 * h + tt * 512
            if tt % 2 == 0:
                nc.scalar.copy(offmA[:, tg : tg + 512], ps[:])
            else:
                v.tensor_copy(offmA[:, tg : tg + 512], ps[:])

    # ---- 4. shuffle offsets into [128, 96]: offP[64h+m, 32k+i] = off[k, t_g]
    offP = smallp.tile([128, 96], F32, name=f"offP{b}", tag="offP")
    for k in range(KS):
        nc.scalar.dma_start(
            offP[:, 32 * k : 32 * k + 32],
            offmA[k : k + 1, :].rearrange("o (p i) -> o p i", p=128, i=32),
        )

    if stage <= 2:
        nc.sync.dma_start(out_d[b, :, 0:96], offP[:])
        return

    # ---- 5. elementwise chain -> coef (f16) and idx (i16, padded to 128)
    p_t = smallp.tile([128, 96], F32, name=f"p{b}", tag="p")
    fl_t = smallp.tile([128, 96], F32, name=f"fl{b}", tag="fl")
    f_t = smallp.tile([128, 96], F32, name=f"f{b}", tag="f")
    m_t = smallp.tile([128, 96], F32, name=f"m{b}", tag="m")
    coef_t = smallp.tile([128, 96], F32, name=f"coef{b}", tag="coef")
    cf16 = smallp.tile([128, 96], F16, name=f"cf16{b}", tag="cf16")
    u1_t = smallp.tile([128, 96], F32, name=f"u1{b}", tag="u1")
    idx_t = smallp.tile([128, 128], I16, name=f"idx{b}", tag="idx")

    v.tensor_add(p_t[:], offP[:], base_sb[:])
    v.tensor_scalar(fl_t[:], p_t[:], 0.5, MAGIC, AluOpType.subtract, AluOpType.add)
    v.tensor_scalar(fl_t[:], fl_t[:], MAGIC, None, AluOpType.subtract)
    v.tensor_sub(f_t[:], p_t[:], fl_t[:])
    # t1 = (f - 1) * f = -f(1-f); two fused mask multiplies; negate in cast
    v.scalar_tensor_tensor(
        coef_t[:], f_t[:], 1.0, f_t[:], AluOpType.subtract, AluOpType.mult
    )
    v.scalar_tensor_tensor(
        coef_t[:], p_t[:], 1.0, coef_t[:], AluOpType.is_ge, AluOpType.mult
    )
    v.scalar_tensor_tensor(
        coef_t[:], p_t[:], 4096.0, coef_t[:], AluOpType.is_le, AluOpType.mult
    )
    v.tensor_scalar(cf16[:], coef_t[:], -1.0, None, AluOpType.mult)
    # idx_local = clip(fl - 2032*(h==1), 0, NE-1) as int16
    v.tensor_scalar(
        u1_t[:], fl_t[:], hoff_sb[:], 0.0, AluOpType.subtract, AluOpType.max
    )
    v.tensor_scalar(idx_t[:, 0:96], u1_t[:], float(NE - 1), None, AluOpType.min)
    # pad cols [96:128) (feeds unused transpose rows); sourced from u1 so the
    # write has a real dependency and cannot be hoisted across slot reuse.
    v.tensor_scalar(idx_t[:, 96:128], u1_t[:, 0:32], 0.0, None, AluOpType.mult)

    # ---- 6. wrapped gather index list ------------------------------------
    # T = idx_t transposed (16-bit XBAR): T[32k+16u+rr, 64h+m] = idx
    T = smallp.tile([128, 128], I16, name=f"T{b}", tag="T")
    nc.sync.dma_start(T[:], idx_t[:], transpose=True)
    nc.sync.dma_start(T_d[b], T[:])
    # list_d[b, h, rr, 128k+64u+m] = T[32k+16u+rr, 64h+m]  (per-core list,
    # unreplicated). DRAM->DRAM: SBUF partition-split APs are mis-tracked.
    for h in range(2):
        for k in range(KS):
            s3 = T_d[b].rearrange(
                "(k2 u r) m -> k2 u r m", k2=4, u=2, r=16
            )[k, :, :, 64 * h : 64 * h + 64]
            d3 = list_d[b, h].rearrange(
                "r (k2 u m) -> k2 u r m", k2=KS, u=2, m=64
            )[k]
            (nc.sync if (h + k) % 2 else nc.scalar).dma_start(d3, s3)
    # replicate the per-core list across the 4 cores of each half (in DRAM:
    # SBUF-side partition-split write APs are mis-tracked by the dep
    # machinery), then load as one plain 2D DMA.
    for h in range(2):
        nc.sync.dma_start(
            wrD[b, 64 * h : 64 * h + 64, :].rearrange("(j r) s -> j r s", j=4, r=16),
            _bc(list_d[b, h], 4),
        )
    wr = smallp.tile([128, NIDX // 16], I16, name=f"wr{b}", tag="wr")
    nc.sync.dma_start(wr[:], wrD[b])

    # ---- 7. replicated coef: crep[64h+c, 2048k+1024u+16m+rr] = coef ------
    nc.sync.dma_start(cf_d[b], cf16[:])
    # cfq_d[b, h, q] = coef(k, t_l(m,u,rr), h),  q = 2048k + 1024u + 16m + rr
    for h in range(2):
        for u in range(2):
            s3 = cf_d[b, 64 * h : 64 * h + 64, :].rearrange(
                "m (k u2 r) -> u2 k m r", k=KS, u2=2, r=16
            )[u]
            d3 = cfq_d[b, h].rearrange(
                "(k u2 m r) -> u2 k m r", k=KS, u2=2, m=64, r=16
            )[u]
            (nc.sync if (h + u) % 2 else nc.scalar).dma_start(d3, s3)
    crep = crepp.tile([128, NIDX], F16, name=f"crep{b}", tag="crep")
    for h in range(2):
        nc.scalar.dma_start(crep[64 * h : 64 * h + 64, :], _bc(cfq_d[b, h], 64))

    if stage <= 3:
        nc.sync.dma_start(out_d[b, :, 0:96], u1_t[:])
        return

    # ---- 8. gather on GPSIMD ---------------------------------------------
    G = gatp.tile([128, NIDX], F32, name=f"G{b}", tag="G", bufs=1)
    nc.gpsimd.ap_gather(
        G[:, :, None],
        xs2[:, :, None],
        wr[:],
        channels=128,
        num_elems=NE,
        d=1,
        num_idxs=NIDX,
    )

    # ---- 9. scale ---------------------------------------------------------
    # separate F32R output tile: the walrus verifier requires every writer
    # of a fp32r-matmul operand's memory to produce float32r, so the raw
    # gather output (plain f32 -- f32r input wedges the gather ucode) and the
    # rounded scaled values must live in different tiles.
    G2 = gatp.tile([128, NIDX], F32R, name=f"G2{b}", tag="G2", bufs=2)
    v.tensor_mul(G2[:], G[:], crep[:])

    if stage <= 4:
        nc.sync.dma_start(out_d[b, :, 0:4096], G2[:, 0:4096].bitcast(F32))
        return

    # ---- 10. main conv (q-ordered psum), un-permute in copies, DMA out ---
    for h in range(2):
        pl = 64 * h
        for pair in range(2):           # pair a: q-tiles tt'=a and a+2
            pms = []
            for uu in range(2):
                tt = 2 * uu + pair
                pm = psm.tile(
                    [O, 512], F32, name=f"psm{b}_{h}_{tt}", tag=f"psm{h}", bufs=2
                )
                for k in range(KS):
                    qb = 2048 * k + 512 * tt
                    nc.tensor.matmul(
                        pm[:],
                        wk_r[pl : pl + 64, k, :],
                        G2[pl : pl + 64, qb : qb + 512],
                        start=(k == 0),
                        stop=(k == KS - 1),
                        tile_position=(pl, 0),
                    )
                pms.append(pm)
            osb = outp.tile([O, 1024], F32, name=f"osb{b}_{h}_{pair}", tag=f"osb{h}")
            for half in range(2):       # output t-tile c = 2*pair + half
                for uu in range(2):
                    # psum col n = 256*half + 16*m' + rr
                    #   -> osb col 512*half + 32*m' + 16*uu + rr
                    src = pms[uu][:, 256 * half : 256 * half + 256].rearrange(
                        "o (m r) -> o m r", m=16, r=16
                    )
                    dst = osb[:, 512 * half : 512 * half + 512].rearrange(
                        "o (m w r) -> o m w r", m=16, w=2, r=16
                    )[:, :, uu]
                    if (half + uu) % 2 == 0:
                        v.tensor_copy(dst, src)
                    else:
                        nc.scalar.copy(dst, src)
            tg = 2048 * h + 1024 * pair
            nc.sync.dma_start(out_d[b, :, tg : tg + 1024], osb[:])


def build_program(stage=9, repeat=1):
    nc = bacc.Bacc("TRN2", target_bir_lowering=False, debug=False)
    x_d = nc.dram_tensor("x8", [BPC, C, L], F32, kind="ExternalInput").ap()
    wo_d = nc.dram_tensor("wo_dup", [128, KS, KS], F32, kind="ExternalInput").ap()
    wk_d = nc.dram_tensor("wk_dup", [128, KS, O], F32, kind="ExternalInput").ap()
    base_d = nc.dram_tensor("base_c", [128, 96], F32, kind="ExternalInput").ap()
    hoff_d = nc.dram_tensor("hoff_c", [128, 1], F32, kind="ExternalInput").ap()
    out_d = nc.dram_tensor("out8", [BPC, O, L], F32, kind="ExternalOutput").ap()
    T_d = nc.dram_tensor("T_d", [BPC, 128, 128], I16, kind="Internal").ap()
    cf_d = nc.dram_tensor("cf_d", [BPC, 128, 96], F16, kind="Internal").ap()
    cfq_d = nc.dram_tensor("cfq_d", [BPC, 2, NIDX], F16, kind="Internal").ap()
    list_d = nc.dram_tensor("list_d", [BPC, 2, 16, NIDX // 16], I16, kind="Internal").ap()
    wrD = nc.dram_tensor("wrD", [BPC, 128, NIDX // 16], I16, kind="Internal").ap()

    with tile.TileContext(nc) as t:
        with (
            t.tile_pool(name="const", bufs=1) as constp,
            t.tile_pool(name="xp", bufs=2) as xpp,
            t.tile_pool(name="xs2", bufs=2) as xs2p,
            t.tile_pool(name="gat", bufs=2) as gatp,
            t.tile_pool(name="crep", bufs=2) as crepp,
            t.tile_pool(name="small", bufs=2) as smallp,
            t.tile_pool(name="outsb", bufs=2) as outp,
            t.tile_pool(name="psum_off", bufs=2, space="PSUM") as pso,
            t.tile_pool(name="psum_main", bufs=2, space="PSUM") as psm,
        ):
            wo_sb = constp.tile([128, KS, KS], F32, name="wo_sb")
            wk_sb = constp.tile([128, KS, O], F32, name="wk_sb")
            base_sb = constp.tile([128, 96], F32, name="base_sb")
            hoff_sb = constp.tile([128, 1], F32, name="hoff_sb")
            nc.sync.dma_start(wo_sb[:], wo_d)
            nc.sync.dma_start(wk_sb[:], wk_d)
            nc.sync.dma_start(base_sb[:], base_d)
            nc.sync.dma_start(hoff_sb[:], hoff_d)
            wo_r = constp.tile([128, KS, KS], F32R, name="wo_r")
            wk_r = constp.tile([128, KS, O], F32R, name="wk_r")
            nc.vector.tensor_copy(wo_r[:], wo_sb[:])
            nc.vector.tensor_copy(wk_r[:], wk_sb[:])
            pools = (xpp, xs2p, gatp, crepp, smallp, outp, pso, psm)
            consts = (
                wo_r, wk_r, base_sb, hoff_sb, x_d, out_d,
                T_d, cf_d, cfq_d, list_d, wrD,
            )
            for _r in range(repeat):
                for b in range(BPC):
                    _emit_sample(nc, b, pools, consts, stage=stage)
    nc.compile()
    return nc


def get_program():
    global _PROGRAM
    if _PROGRAM is None:
        _PROGRAM = build_program()
    return _PROGRAM


def host_inputs(x, w_off, b_off, w_conv):
    """Pure layout prep of the (runtime) inputs -> per-core in_maps."""
    x = np.ascontiguousarray(np.asarray(x, dtype=np.float32))
    w_off = np.asarray(w_off, dtype=np.float32)
    b_off = np.asarray(b_off, dtype=np.float32)
    w_conv = np.asarray(w_conv, dtype=np.float32)

    wo_half = np.transpose(w_off, (1, 2, 0))          # [c, j, k]
    wo_dup = np.ascontiguousarray(np.concatenate([wo_half, wo_half], axis=0))
    wk_half = np.transpose(w_conv, (1, 2, 0))         # [c, k, o]
    wk_dup = np.ascontiguousarray(np.concatenate([wk_half, wk_half], axis=0))

    p = np.arange(128)
    i = np.arange(32)
    k = np.arange(KS)
    t_g = (32 * p)[:, None, None] + i[None, None, :]                  # [128,1,32]
    base = t_g + k[None, :, None] + b_off[None, :, None]              # [128,3,32]
    base_c = np.ascontiguousarray(base.reshape(128, 96).astype(np.float32))
    hoff_c = np.where(p >= 64, float(H1OFF), 0.0).astype(np.float32)[:, None]
    hoff_c = np.ascontiguousarray(hoff_c)

    in_maps = []
    for core in range(N_CORES):
        in_maps.append(
            {
                "x8": x[core * BPC : (core + 1) * BPC],
                "wo_dup": wo_dup,
                "wk_dup": wk_dup,
                "base_c": base_c,
                "hoff_c": hoff_c,
            }
        )
    return in_maps


def kernel(x, w_off, b_off, w_conv):
    from concourse import bass_utils

    nc = get_program()
    in_maps = host_inputs(x, w_off, b_off, w_conv)
    res = bass_utils.run_bass_kernel_spmd(
        nc, in_maps, core_ids=list(range(N_CORES))
    )
    out = np.concatenate([r["out8"] for r in res.results], axis=0)
    return out.astype(np.float32)



# revision 2
# speedup vs baseline: 5.7450x; 5.7450x over previous
"""DeformConv1D Trainium2 Bass kernel, v2.

Problem: B=64, C=64, L=4096, OUTC=128, KS=3 deformable conv1d.
Math (validated in the v1 kernel):
    offset[b,k,t] = sum_{c,j} w_off[k,c,j] * xp[b,c,t+j] + b_off[k]
    p = t + k + offset[b,k,t]
    fl = round_half_even(p - 0.5)   (== floor(p) except at integer p, coef=0)
    f  = p - fl
    coef = (1 <= p <= 4096) * f * (1 - f)
    out[b,o,t] = sum_{k,c} w_conv[o,c,k] * coef[b,t,k] * xs2[b,c,fl[b,t,k]]
with xp = zero-pad(x, 1) (length 4098), xs2[u] = xp[u] + xp[u+1].

v2 layout (per core: 8 samples, data-parallel over batch):
  - t split into 8 octants tq (512 t each); GPSIMD core tq handles octant tq.
    Partitions P = 16 tq + r hold channels 4r..4r+3 interleaved along the
    free dim in a 533-wide xp window around the octant (host-prepped fp16):
        xw[P, u_w, c4] = xp[4r+c4, 512 tq - 8 + u_w]
    xsw = xw[:, :532] + xw[:, 1:533]  (one DVE add) is the gather source.
  - ap_gather d=4 fp16: 1536 indices per core list (4x fewer Q7 gather
    requests than the v1 d=1 f32 6144-index gather).
  - index list position i = 512 k + 16 m + rr with t' = m + 32 rr
    (t = 512 tq + t'): the wrapped [16, 96] per-core list is a plain slice
    of the chain tile (partition 16 tq + rr, col 32 k + m) -- no shuffles.
  - coef: one XBAR transpose + one 768-descriptor DRAM store per sample,
    loaded back replicated over r as CF [128, 1536] fp16.
  - main conv: per octant, 12 fp16 matmuls (k, c4) contract 32 partitions
    (the other octant's rows carry zero weights) into a [128, 512] psum;
    psum->SBUF copies un-permute (m, rr) -> t'.
  - offset conv: xpj[64j + c, u] = xp[c, u+j] fold (j in {0,1}):
    offset = wo_a^T xpj[:, t] + wo_b^T xpj[:, t+1], 2 matmuls per 512-tile
    (wo_b is zero in its top 64 rows).
"""

import sys

import numpy as np

sys.path.insert(0, "/opt/trn_rl_repo")

import concourse.bass as bass
import concourse.bacc as bacc
import concourse.mybir as mybir
import concourse.tile as tile
from concourse.alu_op_type import AluOpType

N_CORES = 8
B, C, L, O, KS = 64, 64, 4096, 128, 3
BPC = B // N_CORES
WIN = 532                 # indexable window positions per octant
WSLOP = 8                 # window starts at 512*tq - WSLOP
NIDX = 1536               # gather indices per Q7 core per sample
MAGIC = 12582912.0        # 1.5 * 2^23
F32 = mybir.dt.float32
F16 = mybir.dt.float16
I16 = mybir.dt.int16

_PROGRAM = None


def _emit_load_offconv(nc, b, P, stage=9):
    """Load inputs for sample b, run the offset conv, scatter into offP."""
    v = nc.vector
    # xpj[64j + c, u] = xp[c, u + j]: one DMA, j encoded as +1 elem offset
    xpj = P.xpjp.tile([128, 4099], F16, name=f"xpj{b}", tag="xpj")
    nc.sync.dma_start(xpj[0:64, :], P.xp_d[b, :, 0:4099])
    nc.scalar.dma_start(xpj[64:128, :], P.xp_d[b, :, 1:4100])

    # windowed interleaved xp -> xsw (gather source)
    xw = P.xwp.tile([128, (WIN + 1) * 4], F16, name=f"xw{b}", tag="xw")
    nc.scalar.dma_start(xw[:], P.xw_d[b].rearrange("p w c -> p (w c)"))
    xsw = P.xswp.tile([128, WIN * 4], F16, name=f"xsw{b}", tag="xsw")
    v.tensor_add(xsw[:], xw[:, 0 : WIN * 4], xw[:, 4 : (WIN + 1) * 4])
    P.xsw[b] = xsw

    if stage <= 1:
        nc.sync.dma_start(P.out_d[b, :, 0:1024], xsw[:, 0:2048].bitcast(F32))
        return

    # offset conv: offm[k, t], accumulated from the two j-fold matmuls
    offm = P.smallp.tile([KS, L], F32, name=f"offm{b}", tag="offm", bufs=1)
    for tt in range(8):
        ps = P.pso.tile([KS, 512], F32, name=f"pso{b}_{tt}", tag="pso")
        cb = tt * 512
        nc.tensor.matmul(ps[:], P.wo_sb[:, 0:3], xpj[:, cb : cb + 512],
                         start=True, stop=False)
        nc.tensor.matmul(ps[:], P.wo_sb[:, 3:6], xpj[:, cb + 1 : cb + 513],
                         start=False, stop=True)
        if tt % 2 == 0:
            nc.scalar.copy(offm[:, cb : cb + 512], ps[:])
        else:
            v.tensor_copy(offm[:, cb : cb + 512], ps[:])

    # offP[16tq + rr, b_rel, 32k + m] = offm[k, 512 tq + 32 rr + m]
    offP, b_rel = P.offP_h[b // 4], b % 4
    for k in range(KS):
        eng = nc.scalar if k % 2 else nc.sync
        eng.dma_start(
            offP[:, b_rel, 32 * k : 32 * k + 32],
            offm[k : k + 1, :].rearrange("o (p m) -> o p m", p=128, m=32),
        )


def _emit_chain(nc, half, P, stage=9):
    """Elementwise chain for samples [4*half, 4*half+4): coef + idx tiles."""
    v = nc.vector
    offP = P.offP_h[half][:].rearrange("p b f -> p (b f)")
    base = P.base_sb[:, 0:384]
    idx_h = P.idx_h[half]
    cfpad = P.cfpad_h[half]
    S = P.smallp
    t_p = S.tile([128, 384], F32, name=f"p{half}", tag="c_p", bufs=1)
    t_fl = S.tile([128, 384], F32, name=f"fl{half}", tag="c_fl", bufs=1)
    t_f = S.tile([128, 384], F32, name=f"f{half}", tag="c_f", bufs=1)
    t_cf = S.tile([128, 384], F32, name=f"cf{half}", tag="c_cf", bufs=1)
    t_uw = S.tile([128, 384], F32, name=f"uw{half}", tag="c_uw", bufs=1)

    v.tensor_add(t_p[:], offP, base)
    v.tensor_scalar(t_fl[:], t_p[:], 0.5, MAGIC, AluOpType.subtract, AluOpType.add)
    v.tensor_scalar(t_fl[:], t_fl[:], MAGIC, None, AluOpType.subtract)
    v.tensor_sub(t_f[:], t_p[:], t_fl[:])
    v.scalar_tensor_tensor(
        t_cf[:], t_f[:], 1.0, t_f[:], AluOpType.subtract, AluOpType.mult
    )
    v.scalar_tensor_tensor(
        t_cf[:], t_p[:], 1.0, t_cf[:], AluOpType.is_ge, AluOpType.mult
    )
    v.scalar_tensor_tensor(
        t_cf[:], t_p[:], 4096.0, t_cf[:], AluOpType.is_le, AluOpType.mult
    )
    # cfpad[:, b_rel, 0:96] = -t_cf (fp16); cols 96:128 hold zeros
    v.tensor_scalar(
        cfpad[:].rearrange("p b w -> p (b w)").rearrange(
            "p (b w) -> p b w", b=4, w=128)[:, :, 0:96],
        t_cf[:].rearrange("p (b f) -> p b f", b=4, f=96),
        -1.0, None, AluOpType.mult,
    )
    # u_w = clip(fl - (512 tq - 8), 0, WIN-1) as int16
    v.tensor_scalar(
        t_uw[:], t_fl[:], P.winb_sb[:], 0.0, AluOpType.subtract, AluOpType.max
    )
    v.tensor_scalar(idx_h[:], t_uw[:], float(WIN - 1), None, AluOpType.min)


def _emit_coef(nc, b, P):
    """XBAR transpose of the coef pad + rearranged DRAM store for sample b."""
    half, b_rel = b // 4, b % 4
    T = P.smallp.tile([128, 128], I16, name=f"T{b}", tag="T")
    nc.sync.dma_start(T[:], P.cfpad_h[half][:, b_rel].bitcast(I16),
                      transpose=True)
    # cf_d[b][tq, 512 k + 16 m + rr] = T[32 k + m, 16 tq + rr]
    nc.scalar.dma_start(
        P.cf_d[b].rearrange("tq (k m rr) -> (k m) tq rr", k=KS, m=32, rr=16),
        T[0:96].rearrange("p (tq rr) -> p tq rr", tq=8, rr=16).bitcast(F16),
    )


def _emit_cf4(nc, half, P):
    """CF4[16tq + r, b_rel, i] = coef(b, tq, i): 8 broadcast loads."""
    CF4 = P.cfp.tile([128, 4, NIDX], F16, name=f"CF4_{half}", tag="CF4")
    a = P.cf_d[4 * half]          # [8, NIDX] slice of [BPC, 8, NIDX]
    bstride = 8 * NIDX
    for tq in range(8):
        src = bass.AP(a.tensor, a.offset + tq * NIDX,
                      [[0, 16], [bstride, 4], [1, NIDX]])
        eng = nc.scalar if tq % 2 else nc.sync
        eng.dma_start(CF4[16 * tq : 16 * tq + 16], src)
    P.CF4_h[half] = CF4


def _emit_gather_conv(nc, b, P, stage=9):
    v = nc.vector
    half, b_rel = b // 4, b % 4

    if stage <= 3:
        nc.sync.dma_start(P.out_d[b, :, 0:96],
                          P.idx_h[half][:, b_rel * 96 : b_rel * 96 + 96])
        return

    # ---- gather ----------------------------------------------------------
    G = P.gatp.tile([128, NIDX, 4], F16, name=f"G{b}", tag="G")
    nc.gpsimd.ap_gather(
        G[:],
        P.xsw[b][:].rearrange("p (w c) -> p w c", c=4),
        P.idx_h[half][:, b_rel * 96 : b_rel * 96 + 96],
        channels=128,
        num_elems=WIN,
        d=4,
        num_idxs=NIDX,
    )
    # ---- scale in place: G *= CF4[:, b_rel] (broadcast over c4) ----------
    ca = P.CF4_h[half][:, b_rel]
    cfb = bass.AP(ca.tensor, ca.offset,
                  [list(ca.ap[0]), list(ca.ap[1]), [0, 4]])
    v.tensor_mul(G[:], G[:], cfb)
    M = G

    if stage <= 4:
        nc.sync.dma_start(P.out_d[b, :, 0:2048], M[:, 0:1024].bitcast(F32))
        return

    # ---- main conv -------------------------------------------------------
    for pair in range(4):           # two octants per output tile
        osb = P.outp.tile([O, 1024], F32, name=f"osb{b}_{pair}", tag="osb")
        for sub in range(2):
            tq = 2 * pair + sub
            q2, par = tq // 2, tq % 2
            pm = P.psm.tile([O, 512], F32, name=f"pm{b}_{tq}", tag="pm")
            pl = 32 * q2
            n = 0
            for k in range(KS):
                for c4 in range(4):
                    nc.tensor.matmul(
                        pm[:],
                        P.wc_sb[pl : pl + 32, 12 * par + 4 * k + c4],
                        M[pl : pl + 32, 512 * k : 512 * k + 512, c4],
                        start=(n == 0),
                        stop=(n == 11),
                        tile_position=(pl, 0),
                    )
                    n += 1
            # un-permute: psum col 16 m + rr -> t' = m + 32 rr
            src = pm[:].rearrange("o (m rr) -> o rr m", m=32, rr=16)
            dst = osb[:, 512 * sub : 512 * sub + 512].rearrange(
                "o (rr m) -> o rr m", rr=16, m=32
            )
            if sub == 0:
                v.tensor_copy(dst, src)
            else:
                nc.scalar.copy(dst, src)
        eng = nc.scalar if pair % 2 else nc.sync
        eng.dma_start(P.out_d[b, :, 1024 * pair : 1024 * pair + 1024],
                      osb[:])


class _Ctx:
    pass


def build_program(stage=9, repeat=1):
    nc = bacc.Bacc("TRN2", target_bir_lowering=False, debug=False)
    P = _Ctx()
    P.xp_d = nc.dram_tensor("xp8", [BPC, C, 4100], F16, kind="ExternalInput").ap()
    P.xw_d = nc.dram_tensor("xw8", [BPC, 128, WIN + 1, 4], F16,
                            kind="ExternalInput").ap()
    wo_d = nc.dram_tensor("wo_c", [128, 6], F16, kind="ExternalInput").ap()
    wc_d = nc.dram_tensor("wc_c", [128, 24, O], F16, kind="ExternalInput").ap()
    base_d = nc.dram_tensor("base_c", [128, 384], F32, kind="ExternalInput").ap()
    winb_d = nc.dram_tensor("winb_c", [128, 1], F32, kind="ExternalInput").ap()
    P.out_d = nc.dram_tensor("out8", [BPC, O, L], F32, kind="ExternalOutput").ap()
    P.cf_d = nc.dram_tensor("cf_d", [BPC, 8, NIDX], F16, kind="Internal").ap()

    with tile.TileContext(nc) as t:
        with (
            t.tile_pool(name="const", bufs=1) as constp,
            t.tile_pool(name="xpj", bufs=3) as xpjp,
            t.tile_pool(name="xw", bufs=2) as xwp,
            t.tile_pool(name="xsw", bufs=5) as xswp,
            t.tile_pool(name="gat", bufs=2) as gatp,
            t.tile_pool(name="cf", bufs=2) as cfp,
            t.tile_pool(name="small", bufs=2) as smallp,
            t.tile_pool(name="outsb", bufs=2) as outp,
            t.tile_pool(name="psum_off", bufs=2, space="PSUM") as pso,
            t.tile_pool(name="psum_main", bufs=3, space="PSUM") as psm,
        ):
            P.xpjp, P.xwp, P.xswp, P.gatp = xpjp, xwp, xswp, gatp
            P.cfp, P.smallp, P.outp, P.pso, P.psm = cfp, smallp, outp, pso, psm

            P.wo_sb = constp.tile([128, 6], F16, name="wo_sb")
            P.wc_sb = constp.tile([128, 24, O], F16, name="wc_sb")
            P.base_sb = constp.tile([128, 384], F32, name="base_sb")
            P.winb_sb = constp.tile([128, 1], F32, name="winb_sb")
            nc.sync.dma_start(P.wo_sb[:], wo_d)
            nc.sync.dma_start(P.wc_sb[:], wc_d)
            nc.sync.dma_start(P.base_sb[:], base_d)
            nc.sync.dma_start(P.winb_sb[:], winb_d)

            for _r in range(repeat):
                P.xsw = {}
                P.offP_h, P.idx_h, P.cfpad_h, P.CF4_h = {}, {}, {}, {}
                for half in range(2):
                    P.offP_h[half] = smallp.tile(
                        [128, 4, 96], F32, name=f"offP{half}", tag=f"offP{half}", bufs=1)
                    P.idx_h[half] = smallp.tile(
                        [128, 384], I16, name=f"idx{half}", tag=f"idx{half}", bufs=1)
                    P.cfpad_h[half] = smallp.tile(
                        [128, 4, 128], F16, name=f"cfp{half}", tag=f"cfp{half}", bufs=1)
                    nc.vector.memset(P.cfpad_h[half][:], 0.0)
                    bs = list(range(half * 4, half * 4 + 4))
                    for b in bs:
                        _emit_load_offconv(nc, b, P, stage=stage)
                    if stage >= 2:
                        _emit_chain(nc, half, P, stage=stage)
                        for b in bs:
                            _emit_coef(nc, b, P)
                        if stage >= 4:
                            _emit_cf4(nc, half, P)
                        for b in bs:
                            _emit_gather_conv(nc, b, P, stage=stage)
    nc.compile()
    return nc


def get_program():
    global _PROGRAM
    if _PROGRAM is None:
        _PROGRAM = build_program()
    return _PROGRAM


def host_inputs(x, w_off, b_off, w_conv):
    x = np.ascontiguousarray(np.asarray(x, dtype=np.float32))
    w_off = np.asarray(w_off, dtype=np.float32)
    b_off = np.asarray(b_off, dtype=np.float32)
    w_conv = np.asarray(w_conv, dtype=np.float32)

    # padded x, fp16: xp[c, u] for u in [0, 4100): col 0 zero, 1..4096 = x
    xp = np.zeros((B, C, 4100), np.float16)
    xp[:, :, 1:4097] = x.astype(np.float16)

    # windowed interleave: xw[b, 16tq + r, u_w, c4] = xp_ext[4r+c4, 512tq-8+u_w]
    xp_ext = np.zeros((B, C, WSLOP + 4100 + WIN), np.float16)
    xp_ext[:, :, WSLOP : WSLOP + 4100] = xp
    starts = (512 * np.arange(8))[:, None] + np.arange(WIN + 1)[None, :]
    xw = xp_ext[:, :, starts]                    # [B, C, 8, WIN+1]
    xw = xw.reshape(B, 16, 4, 8, WIN + 1)        # c = 4r + c4 -> (r, c4)
    xw = np.ascontiguousarray(np.transpose(xw, (0, 3, 1, 4, 2)))
    xw = xw.reshape(B, 128, WIN + 1, 4)

    # offset conv weights fold: wo[64j + c, k] = w_off[k, c, j] (j in {0,1});
    # wo[:, 3:6]: j=2 weights on the lower 64 rows only
    wo = np.zeros((128, 6), np.float16)
    for j in range(2):
        wo[64 * j : 64 * j + 64, 0:3] = np.transpose(w_off[:, :, j])
    wo[64:128, 3:6] = np.transpose(w_off[:, :, 2])

    # main conv weights: [128, 24, O] fp16
    # wc[32 q2 + p32, 12 par + 4k + c4, o] =
    #     w_conv[o, 4 (p32 - 16 par) + c4, k] if p32 in [16par, 16par+16) else 0
    wc = np.zeros((128, 24, O), np.float16)
    wt = np.transpose(w_conv, (1, 2, 0))  # [c, k, o]
    for par in range(2):
        for k in range(KS):
            for c4 in range(4):
                blk = np.zeros((32, O), np.float32)
                blk[16 * par : 16 * par + 16] = wt[c4::4, k]
                for q2 in range(4):
                    wc[32 * q2 : 32 * q2 + 32, 12 * par + 4 * k + c4] = blk
    # base[16 tq + rr, (b_rel, 32k + m)] = t + k + b_off[k], t = 512tq+32rr+m
    tq = np.arange(8)
    rr = np.arange(16)
    kk = np.arange(KS)
    m = np.arange(32)
    t = (512 * tq[:, None, None, None] + 32 * rr[None, :, None, None]
         + m[None, None, None, :])
    bse = t + kk[None, None, :, None] + b_off[None, None, :, None]
    bse = bse.reshape(128, 96).astype(np.float32)
    base = np.ascontiguousarray(np.tile(bse, (1, 4)))

    winb = np.repeat((512.0 * np.arange(8) - WSLOP), 16).astype(np.float32)
    winb = np.ascontiguousarray(winb[:, None])

    in_maps = []
    for core in range(N_CORES):
        sl = slice(core * BPC, (core + 1) * BPC)
        in_maps.append({
            "xp8": np.ascontiguousarray(xp[sl]),
            "xw8": np.ascontiguousarray(xw[sl]),
            "wo_c": np.ascontiguousarray(wo),
            "wc_c": np.ascontiguousarray(wc),
            "base_c": base,
            "winb_c": winb,
        })
    return in_maps


def kernel(x, w_off, b_off, w_conv):
    from concourse import bass_utils

    nc = get_program()
    in_maps = host_inputs(x, w_off, b_off, w_conv)
    res = bass_utils.run_bass_kernel_spmd(
        nc, in_maps, core_ids=list(range(N_CORES))
    )
    out = np.concatenate([r["out8"] for r in res.results], axis=0)
    return out.astype(np.float32)
